# revision 20
# baseline (speedup 1.0000x reference)
"""Trainium2 Bass kernel for nn_Attention_86268713108190.

7 independent attention "bands" over batch 8, n=512, d=512, 8 heads,
shared Wqkv/Wout. Sharding: data-parallel over batch — core c handles
batch index c (7 band-samples of [512, 512] each).

Per-core dataflow (per sample, all matmuls in float32r):
  1. qkvT = Wqkv @ x^T    (lhsT = WqkvT chunks, rhs = x^T)      [e, n]
  2. v    = x @ Wv^T      (lhsT = x^T chunks,   rhs = WvT)      [n, ev]
     v_aug: per head 64 v-cols + a ones column (65) -> denominator for free
  3. per head: S^T = k_h q_h^T (K=64, two heads packed via tile_position),
     expS^T = exp(SCALE*S^T) on ACT (PSUM->SBUF, rounds to f32r),
     O_aug^T[65, n] = AV matmul accumulating over j; row 64 = denominator.
  4. recip denominators (DVE), partition-broadcast (GpSimd),
     normalize O^T (DVE) -> OT in [d, n] layout.
  5. out = O @ Wout^T + bias  (lhsT = OT chunks, rhs = WoutT).
"""

import contextlib
import sys

if '/opt/trn_rl_repo' not in sys.path:
    sys.path.insert(0, '/opt/trn_rl_repo')

import numpy as np

P = 128
MM_DTYPE = "f32r"
NSEQ = 512
D = 512
H = 8
DH = 64
NBANDS = 7
NCORES = 8
SCALE = D ** -0.5

_cached = None


def _emit_band(ctx, s, xt):
    """Emit one band's compute. `xt` is the (already DMA'd) x^T tile."""
    nc, f32, f32r, Exp = ctx["nc"], ctx["f32"], ctx["f32r"], ctx["Exp"]
    wq_sb, wo_sb, bias_sb = ctx["wq_sb"], ctx["wo_sb"], ctx["bias_sb"]
    out = ctx["out"]
    pl = ctx["pools"]

    # --- QKV projections -> qkvT layout for q,k ---
    qk_sb = pl["qk"].tile([P, 8, NSEQ], f32r, tag="qk")
    for et in (0, 4, 1, 5, 2, 6, 3, 7):
        ps = pl["psproj"].tile([P, NSEQ], f32, tag="psproj")
        for kt in range(4):
            nc.tensor.matmul(
                ps[:], wq_sb[:, kt, et * P:(et + 1) * P], xt[:, kt, :],
                start=(kt == 0), stop=(kt == 3))
        nc.vector.tensor_copy(qk_sb[:, et, :], ps[:])

    # --- V projection -> row-major v_aug with ones column ---
    v_aug = pl["v"].tile([P, 4, H, DH + 1], f32r, tag="vaug")
    for nt in range(4):
        ps = pl["psproj"].tile([P, NSEQ], f32, tag="psproj")
        for kt in range(4):
            nc.tensor.matmul(
                ps[:], xt[:, kt, nt * P:(nt + 1) * P],
                wq_sb[:, kt, 2 * D:3 * D],
                start=(kt == 0), stop=(kt == 3))
        nc.vector.tensor_copy(
            v_aug[:, nt, :, 0:DH],
            ps[:].rearrange("p (h dh) -> p h dh", h=H))
        ones_slice = v_aug[:, nt, :, DH:DH + 1]
        if ctx["mm_dtype"] == "f32r":
            ones_slice = ones_slice.bitcast(f32)
        nc.vector.memset(ones_slice, 1.0)

    # --- attention per head pair (2g, 2g+1), software-pipelined ---
    # PE stream: S(0), S(1), AV(0), S(2), AV(1), S(3), AV(2), AV(3)
    # so PE never stalls waiting for ACT's exp of the same pair.
    ot_sb = pl["ot"].tile([P, 4, NSEQ], f32r, tag="ot")
    es_store = {}

    def s_phase(g):
        es_list = []
        for jt in range(4):
            ps_s0 = pl["pss"].tile([P, NSEQ], f32, tag="pss")
            ps_s1 = pl["pss"].tile([P, NSEQ], f32, tag="pss")
            nc.tensor.matmul(
                ps_s0[:],
                qk_sb[0:DH, 4 + g, jt * P:(jt + 1) * P],
                qk_sb[0:DH, g, :], start=True, stop=True)
            nc.tensor.matmul(
                ps_s1[:],
                qk_sb[DH:P, 4 + g, jt * P:(jt + 1) * P],
                qk_sb[DH:P, g, :], start=True, stop=True,
                tile_position=(DH, 0))
            es = pl["es"].tile([P, 2, NSEQ], f32r, tag="es")
            nc.scalar.activation(es[:, 0, :], ps_s0[:], Exp, scale=SCALE)
            nc.scalar.activation(es[:, 1, :], ps_s1[:], Exp, scale=SCALE)
            es_list.append(es)
        es_store[g] = es_list

    def av_phase(g):
        es_list = es_store.pop(g)
        ps_o0 = pl["pso"].tile([DH + 1, NSEQ], f32, tag="pso")
        ps_o1 = pl["pso"].tile([DH + 1, NSEQ], f32, tag="pso")
        for jt in range(4):
            nc.tensor.matmul(
                ps_o0[:], v_aug[:, jt, 2 * g, :], es_list[jt][:, 0, :],
                start=(jt == 0), stop=(jt == 3))
            nc.tensor.matmul(
                ps_o1[:], v_aug[:, jt, 2 * g + 1, :], es_list[jt][:, 1, :],
                start=(jt == 0), stop=(jt == 3))
        rc0 = pl["r"].tile([1, NSEQ], f32, tag="rc0")
        rc1 = pl["r"].tile([1, NSEQ], f32, tag="rc1")
        nc.vector.reciprocal(rc0[:], ps_o0[DH:DH + 1, :])
        nc.vector.reciprocal(rc1[:], ps_o1[DH:DH + 1, :])
        rb0 = pl["r"].tile([DH, NSEQ], f32, tag="rb0")
        rb1 = pl["r"].tile([DH, NSEQ], f32, tag="rb1")
        nc.gpsimd.partition_broadcast(rb0[:], rc0[:])
        nc.gpsimd.partition_broadcast(rb1[:], rc1[:])
        nc.vector.tensor_mul(ot_sb[0:DH, g, :], ps_o0[0:DH, :], rb0[:])
        nc.vector.tensor_mul(ot_sb[DH:P, g, :], ps_o1[0:DH, :], rb1[:])

    def interleaved_pair(g):
        ps_o0 = pl["pso"].tile([DH + 1, NSEQ], f32, tag="pso")
        ps_o1 = pl["pso"].tile([DH + 1, NSEQ], f32, tag="pso")
        for jt in range(4):
            ps_s0 = pl["pss"].tile([P, NSEQ], f32, tag="pss")
            ps_s1 = pl["pss"].tile([P, NSEQ], f32, tag="pss")
            nc.tensor.matmul(
                ps_s0[:],
                qk_sb[0:DH, 4 + g, jt * P:(jt + 1) * P],
                qk_sb[0:DH, g, :], start=True, stop=True)
            nc.tensor.matmul(
                ps_s1[:],
                qk_sb[DH:P, 4 + g, jt * P:(jt + 1) * P],
                qk_sb[DH:P, g, :], start=True, stop=True,
                tile_position=(DH, 0))
            es = pl["es"].tile([P, 2, NSEQ], f32r, tag="es")
            nc.scalar.activation(es[:, 0, :], ps_s0[:], Exp, scale=SCALE)
            nc.scalar.activation(es[:, 1, :], ps_s1[:], Exp, scale=SCALE)
            nc.tensor.matmul(
                ps_o0[:], v_aug[:, jt, 2 * g, :], es[:, 0, :],
                start=(jt == 0), stop=(jt == 3))
            nc.tensor.matmul(
                ps_o1[:], v_aug[:, jt, 2 * g + 1, :], es[:, 1, :],
                start=(jt == 0), stop=(jt == 3))
        rc0 = pl["r"].tile([1, NSEQ], f32, tag="rc0")
        rc1 = pl["r"].tile([1, NSEQ], f32, tag="rc1")
        nc.vector.reciprocal(rc0[:], ps_o0[DH:DH + 1, :])
        nc.vector.reciprocal(rc1[:], ps_o1[DH:DH + 1, :])
        rb0 = pl["r"].tile([DH, NSEQ], f32, tag="rb0")
        rb1 = pl["r"].tile([DH, NSEQ], f32, tag="rb1")
        nc.gpsimd.partition_broadcast(rb0[:], rc0[:])
        nc.gpsimd.partition_broadcast(rb1[:], rc1[:])
        nc.vector.tensor_mul(ot_sb[0:DH, g, :], ps_o0[0:DH, :], rb0[:])
        nc.vector.tensor_mul(ot_sb[DH:P, g, :], ps_o1[0:DH, :], rb1[:])

    if ctx["pipe"] == "pipe":
        s_phase(0)
        for g in range(1, 4):
            s_phase(g)
            av_phase(g - 1)
        av_phase(3)
    elif ctx["pipe"] == "split":
        for g in range(4):
            s_phase(g)
            av_phase(g)
    else:  # "v2": exp and AV interleaved per j-tile
        for g in range(4):
            interleaved_pair(g)

    # --- output projection + bias ---
    for nt in range(4):
        ps = pl["psproj"].tile([P, NSEQ], f32, tag="psproj")
        for kt in range(4):
            nc.tensor.matmul(
                ps[:], ot_sb[:, kt, nt * P:(nt + 1) * P], wo_sb[:, kt, :],
                start=(kt == 0), stop=(kt == 3))
        ob = pl["ob"].tile([P, D], f32, tag="ob")
        nc.vector.tensor_add(ob[:], ps[:], bias_sb[:])
        nc.sync.dma_start(
            out[s].rearrange("(no ni) e -> ni no e", ni=P)[:, nt, :], ob[:])


def build_kernel(nbands=NBANDS, repeat=1, mm_dtype=MM_DTYPE, pipe="split"):
    import concourse.mybir as mybir
    import concourse.tile as tile
    from concourse import bacc
    from concourse import library_config

    f32 = mybir.dt.float32
    f32r = (mybir.dt.float32r if mm_dtype == "f32r" else mybir.dt.bfloat16)
    Exp = mybir.ActivationFunctionType.Exp

    nc = bacc.Bacc("TRN2", target_bir_lowering=False, debug=False,
                   num_devices=NCORES)

    xT = nc.dram_tensor("xT", [nbands, D, NSEQ], f32r, kind="ExternalInput").ap()
    wqkvT = nc.dram_tensor("wqkvT", [D, 3 * D], f32r, kind="ExternalInput").ap()
    woutT = nc.dram_tensor("woutT", [D, D], f32r, kind="ExternalInput").ap()
    biasb = nc.dram_tensor("biasb", [P, D], f32, kind="ExternalInput").ap()
    out = nc.dram_tensor("out", [nbands, NSEQ, D], f32, kind="ExternalOutput").ap()

    nc.gpsimd.load_library(library_config.attn)

    with tile.TileContext(nc) as tc:
        with (
            tc.tile_pool(name="weights", bufs=1) as wpool,
            tc.tile_pool(name="x", bufs=3) as xpool,
            tc.tile_pool(name="qk", bufs=2) as qkpool,
            tc.tile_pool(name="v", bufs=2) as vpool,
            tc.tile_pool(name="ot", bufs=2) as otpool,
            tc.tile_pool(name="es", bufs=8) as spool,
            tc.tile_pool(name="r", bufs=3) as rpool,
            tc.tile_pool(name="ob", bufs=3) as outpool,
            tc.tile_pool(name="psproj", bufs=2, space="PSUM") as psproj,
            tc.tile_pool(name="pss", bufs=3, space="PSUM") as pss,
            tc.tile_pool(name="pso", bufs=3, space="PSUM") as pso,
        ):
            # weights: split wq by k-chunk so the first matmuls can start
            # as soon as their chunk lands
            wq_sb = wpool.tile([P, 4, 3 * D], f32r)
            wo_sb = wpool.tile([P, 4, D], f32r)
            bias_sb = wpool.tile([P, D], f32)
            wq_r = wqkvT.rearrange("(ko ki) e -> ki ko e", ki=P)
            for kt in range(4):
                nc.sync.dma_start(wq_sb[:, kt, :], wq_r[:, kt, :])
            nc.sync.dma_start(wo_sb[:], woutT.rearrange("(ko ki) e -> ki ko e", ki=P))
            nc.sync.dma_start(bias_sb[:], biasb[:])

            ctx = {
                "nc": nc, "f32": f32, "f32r": f32r, "Exp": Exp,
                "mm_dtype": mm_dtype, "pipe": pipe,
                "wq_sb": wq_sb, "wo_sb": wo_sb, "bias_sb": bias_sb,
                "out": out,
                "pools": {
                    "qk": qkpool, "v": vpool, "ot": otpool, "es": spool,
                    "r": rpool, "ob": outpool, "psproj": psproj,
                    "pss": pss, "pso": pso,
                },
            }

            def load_x(s):
                xt = xpool.tile([P, 4, NSEQ], f32r, tag="xt")
                nc.sync.dma_start(
                    xt[:], xT[s].rearrange("(ko ki) n -> ki ko n", ki=P))
                return xt

            rep_ctx = (tc.For_i(0, repeat, 1,
                                hint_engines=(mybir.EngineType.PE,
                                              mybir.EngineType.Activation,
                                              mybir.EngineType.DVE))
                       if repeat > 1 else contextlib.nullcontext())
            with rep_ctx:
                # prefetch x one band ahead
                xt_next = load_x(0)
                for s in range(nbands):
                    xt = xt_next
                    if s + 1 < nbands:
                        xt_next = load_x(s + 1)
                    _emit_band(ctx, s, xt)

    nc.compile()
    return nc


def _get_nc():
    global _cached
    if _cached is None:
        _cached = build_kernel()
    return _cached


def make_in_maps(x, x_delta, x_theta, x_alpha, x_beta, x_gamma, x_upper,
                 Wqkv, Wout, bout, mm_dtype=MM_DTYPE):
    if mm_dtype == "f32r":
        cast_dt = np.float32
    else:
        import ml_dtypes
        cast_dt = ml_dtypes.bfloat16
    xs = np.stack([np.asarray(a, dtype=np.float32) for a in
                   (x, x_delta, x_theta, x_alpha, x_beta, x_gamma, x_upper)],
                  axis=0)  # [7, b, n, d]
    xsT = np.ascontiguousarray(xs.transpose(1, 0, 3, 2).astype(cast_dt))
    wqkvT = np.ascontiguousarray(np.asarray(Wqkv, np.float32).T.astype(cast_dt))
    woutT = np.ascontiguousarray(np.asarray(Wout, np.float32).T.astype(cast_dt))
    biasb = np.ascontiguousarray(
        np.broadcast_to(np.asarray(bout, np.float32)[None, :], (P, D)))
    return [
        {"xT": xsT[c], "wqkvT": wqkvT, "woutT": woutT, "biasb": biasb}
        for c in range(NCORES)
    ]


def kernel(x, x_delta, x_theta, x_alpha, x_beta, x_gamma, x_upper,
           Wqkv, Wout, bout):
    from concourse.bass_utils import run_bass_kernel_spmd

    nc = _get_nc()
    in_maps = make_in_maps(x, x_delta, x_theta, x_alpha, x_beta, x_gamma,
                           x_upper, Wqkv, Wout, bout)
    res = run_bass_kernel_spmd(nc, in_maps, core_ids=list(range(NCORES)))
    full = np.empty((NBANDS, NCORES, NSEQ, D), dtype=np.float32)
    for c in range(NCORES):
        full[:, c] = res.results[c]["out"]
    return tuple(full[i] for i in range(NBANDS))
